# revision 3
# baseline (speedup 1.0000x reference)
"""Dense dot-product attention (score = Q@V^T, softmax, context = A@V) on 8
TRN2 NeuronCores, batch-parallel: each core owns B/8 = 2 batches.

Structure per core (Lq = Lkv = 1024, D = 512, fp32 I/O):
  Prologue: load Q, V for both batches (natural [l, d] layout); PE-transpose
    all QT/VT blocks (PSUM->SBUF copies round to float32r: 1 cycle/row matmul
    speed with ~13-bit mantissa, measured score RMS err 3e-3); cast V to fp16.
  Flat q-tile loop interleaving the two batches (16 slots), with the score
    matmul emitted two slots ahead of its softmax consumer so the PE never
    drains. Per slot:
      mm1: S = QT.T @ VT into PSUM [128, 1024], per-512 halves; DVE
        reduce_max(negate) per half overlaps the other half's matmuls.
      ACT exp(S - max) per half -> fp16 E with fused row-sum; DMA xbar
        block-transpose per half (dest[p, kt, j] = E[j, kt*128+p]) so the
        second matmul can start after the first half.
      mm2: C = ET.T @ V_fp16 into PSUM; attn = E*(1/Z) (ACT copy+scale),
        context = C*(1/Z) (DVE), stores on the scalar HWDGE ring to keep the
        sync ring free for the xbar transposes.
"""
import sys

sys.path.insert(0, "/opt/trn_rl_repo")

import collections
from contextlib import ExitStack

import numpy as np

import concourse.bass as bass
import concourse.tile as tile
from concourse import mybir
from concourse.bass_utils import run_bass_kernel_spmd

F32 = mybir.dt.float32
F32R = mybir.dt.float32r
F16 = mybir.dt.float16

N_CORES = 8
B, LQ, LKV, D = 16, 1024, 1024, 512
BPC = B // N_CORES  # batches per core
NQT = LQ // 128  # q-tiles per batch
NKT = LKV // 128  # k-tiles per batch
NDT = D // 128  # d-tiles


# --- post-Tile pass: hardware wait-slot limits -------------------------------
# Engine instructions carry a single hardware semaphore-wait slot; Tile's
# sem-assigner sometimes emits more. Hoist excess waits onto single-wait NOPs
# spliced immediately before the instruction on the same engine (the NX
# sequencer dispatches in order, so the NOPs block until the sems clear).
_WAIT_LIMITS = collections.defaultdict(lambda: 1)


def _fix_wait_limits(nc):
    n_fixed = 0
    for fn in nc.m.functions:
        for blk in fn.blocks:
            out = []
            for inst in blk.instructions:
                limit = _WAIT_LIMITS[type(inst).__name__]
                si = inst.sync_info
                if si is not None and si.on_wait and len(si.on_wait) > limit:
                    hoist = list(si.on_wait)[: len(si.on_wait) - limit]
                    keep = list(si.on_wait)[len(si.on_wait) - limit :]
                    for i, w in enumerate(hoist):
                        out.append(
                            mybir.InstNoOp(
                                name=f"{inst.name}-waitnop{i}",
                                engine=inst.engine,
                                sync_info=mybir.SyncInfo(on_wait=[w], on_update=[]),
                                bass_nofuse=True,
                            )
                        )
                    inst.sync_info = mybir.SyncInfo(
                        on_wait=keep, on_update=list(si.on_update or [])
                    )
                    n_fixed += 1
                out.append(inst)
            blk.instructions = out
    return n_fixed


def build():
    nc = bass.Bass("TRN2", target_bir_lowering=False, debug=False)
    q = nc.dram_tensor("query", [BPC, LQ, D], F32, kind="ExternalInput").ap()
    v = nc.dram_tensor("value", [BPC, LKV, D], F32, kind="ExternalInput").ap()
    iden = nc.dram_tensor("iden", [128, 128], F32, kind="ExternalInput").ap()
    ctx_out = nc.dram_tensor("context", [BPC, LQ, D], F32, kind="ExternalOutput").ap()
    attn_out = nc.dram_tensor("attn", [BPC, LQ, LKV], F32, kind="ExternalOutput").ap()

    with ExitStack() as ctx:
        tc = ctx.enter_context(tile.TileContext(nc))
        singles = ctx.enter_context(tc.tile_pool(name="singles", bufs=1))
        iop = ctx.enter_context(tc.tile_pool(name="io", bufs=2))
        tp = ctx.enter_context(tc.tile_pool(name="tp", bufs=2))
        ep = ctx.enter_context(tc.tile_pool(name="ep", bufs=3))
        sp = ctx.enter_context(tc.tile_pool(name="sp", bufs=12))
        # PSUM: S tiles 2 banks x 3 bufs = 6; transpose staging / C-psum share
        # the last 2 banks (prologue vs steady state, temporally disjoint).
        pss = ctx.enter_context(tc.tile_pool(name="pss", bufs=3, space="PSUM"))
        psc = ctx.enter_context(tc.tile_pool(name="psc", bufs=2, space="PSUM"))

        ident = singles.tile([128, 128], F32)
        nc.sync.dma_start(ident[:], iden)

        # ---- prologue: loads ------------------------------------------------
        qn, vn = {}, {}
        for b in range(BPC):
            qn[b] = iop.tile([128, NQT, D], F32, tag="qn", name=f"qn{b}")
            nc.sync.dma_start(qn[b][:], q[b].rearrange("(t p) d -> p t d", p=128))
            vn[b] = iop.tile([128, NKT, D], F32, tag="vn", name=f"vn{b}")
            nc.sync.dma_start(vn[b][:], v[b].rearrange("(t p) d -> p t d", p=128))

        # ---- prologue: all transposes + fp16 V casts ------------------------
        qt, vt, vh = {}, {}, {}
        copy_eng = 0
        for b in range(BPC):
            vh[b] = tp.tile([128, NKT, D], F16, tag="vh", name=f"vh{b}")
            for kt in range(NKT):
                nc.vector.tensor_copy(vh[b][:, kt, :], vn[b][:, kt, :])
            qt[b] = tp.tile([128, NDT, LQ], F32R, tag="qt", name=f"qt{b}")
            vt[b] = tp.tile([128, NDT, LKV], F32R, tag="vt", name=f"vt{b}")
            for src, dst in ((qn[b], qt[b]), (vn[b], vt[b])):
                for dt in range(NDT):
                    for g in range(2):
                        pst = psc.tile([128, 512], F32, tag="pb", name=f"pst{b}{dt}{g}")
                        for j in range(4):
                            blk = g * 4 + j
                            nc.tensor.transpose(
                                pst[:, j * 128 : (j + 1) * 128],
                                src[:, blk, dt * 128 : (dt + 1) * 128],
                                ident[:],
                            )
                        dslice = dst[:, dt, g * 512 : (g + 1) * 512]
                        # alternate copies between DVE and ACT
                        if copy_eng % 2 == 0:
                            nc.vector.tensor_copy(dslice, pst[:])
                        else:
                            nc.scalar.copy(dslice, pst[:])
                        copy_eng += 1

        # ---- steady state: flat q-slot loop, batches interleaved ------------
        slots = [(b, qb) for qb in range(NQT) for b in range(BPC)]
        spsum = {}

        def mm1(slot):
            b, qb = slot
            s = pss.tile([128, LKV], F32, tag="s", name=f"s{b}_{qb}")
            halves = []
            for kc in range(2):
                for dt in range(NDT):
                    nc.tensor.matmul(
                        s[:, kc * 512 : (kc + 1) * 512],
                        qt[b][:, dt, qb * 128 : (qb + 1) * 128],
                        vt[b][:, dt, kc * 512 : (kc + 1) * 512],
                        start=(dt == 0),
                        stop=(dt == NDT - 1),
                    )
                # negated max of this half, overlapping the next half's MMs
                nm = sp.tile([128, 1], F32, tag="nm", name=f"nm{b}_{qb}_{kc}")
                nc.vector.reduce_max(
                    nm[:], s[:, kc * 512 : (kc + 1) * 512],
                    axis=mybir.AxisListType.X, negate=True,
                )
                halves.append(nm)
            spsum[slot] = (s, halves)

        def softmax_mm2(slot):
            b, qb = slot
            s, (nm0, nm1) = spsum.pop(slot)
            nmx = sp.tile([128, 1], F32, tag="nmx", name=f"nmx{b}_{qb}")
            nc.vector.tensor_tensor(nmx[:], nm0[:], nm1[:], op=mybir.AluOpType.min)
            e16 = ep.tile([128, LKV], F16, tag="e16", name=f"e16{b}_{qb}")
            et = ep.tile([128, NKT, 128], F16, tag="et", name=f"et{b}_{qb}")
            zs = []
            for h in range(2):
                z = sp.tile([128, 1], F32, tag="z", name=f"z{b}_{qb}_{h}")
                nc.scalar.activation(
                    e16[:, h * 512 : (h + 1) * 512],
                    s[:, h * 512 : (h + 1) * 512],
                    mybir.ActivationFunctionType.Exp,
                    bias=nmx[:],
                    scale=1.0,
                    accum_out=z[:],
                )
                nc.sync.dma_start(
                    et[:, h * 4 : (h + 1) * 4, :],
                    e16[:, h * 512 : (h + 1) * 512],
                    transpose=True,
                )
                zs.append(z)
            zsum = sp.tile([128, 1], F32, tag="zsum", name=f"zsum{b}_{qb}")
            nc.vector.tensor_add(zsum[:], zs[0][:], zs[1][:])
            rz = sp.tile([128, 1], F32, tag="rz", name=f"rz{b}_{qb}")
            nc.vector.reciprocal(rz[:], zsum[:])
            # context matmul (PE) — emitted before the attn scale/stores
            cps = psc.tile([128, D], F32, tag="pb", name=f"c{b}_{qb}")
            for kt in range(NKT):
                nc.tensor.matmul(
                    cps[:],
                    et[:, kt, :],
                    vh[b][:, kt, :],
                    start=(kt == 0),
                    stop=(kt == NKT - 1),
                )
            # attn = E * (1/Z) on ACT; store via scalar HWDGE ring
            a32 = ep.tile([128, LKV], F32, tag="a32", name=f"a32{b}_{qb}")
            nc.scalar.activation(
                a32[:], e16[:], mybir.ActivationFunctionType.Copy, scale=rz[:]
            )
            nc.scalar.dma_start(attn_out[b, qb * 128 : (qb + 1) * 128, :], a32[:])
            c32 = ep.tile([128, D], F32, tag="c32", name=f"c32{b}_{qb}")
            nc.vector.tensor_scalar_mul(c32[:], cps[:], rz[:])
            nc.scalar.dma_start(ctx_out[b, qb * 128 : (qb + 1) * 128, :], c32[:])

        mm1(slots[0])
        mm1(slots[1])
        for i, slot in enumerate(slots):
            if i + 2 < len(slots):
                mm1(slots[i + 2])
            softmax_mm2(slot)

    _fix_wait_limits(nc)
    return nc


_NC = None


def _get_nc():
    global _NC
    if _NC is None:
        _NC = build()
    return _NC


_IDEN = np.eye(128, dtype=np.float32)


def kernel(query: np.ndarray, value: np.ndarray):
    query = np.ascontiguousarray(query, dtype=np.float32)
    value = np.ascontiguousarray(value, dtype=np.float32)
    nc = _get_nc()
    in_maps = [
        {
            "query": query[c * BPC : (c + 1) * BPC],
            "value": value[c * BPC : (c + 1) * BPC],
            "iden": _IDEN,
        }
        for c in range(N_CORES)
    ]
    res = run_bass_kernel_spmd(nc, in_maps, core_ids=list(range(N_CORES)))
    context = np.concatenate([r["context"] for r in res.results], axis=0)
    attn = np.concatenate([r["attn"] for r in res.results], axis=0)
    return context, attn


# revision 4
# speedup vs baseline: 1.0846x; 1.0846x over previous
"""Dense dot-product attention (score = Q@V^T, softmax, context = A@V) on 8
TRN2 NeuronCores, batch-parallel: each core owns B/8 = 2 batches.

Structure per core (Lq = Lkv = 1024, D = 512, fp32 I/O):
  Prologue: load Q, V for both batches (natural [l, d] layout); PE-transpose
    all QT/VT blocks (PSUM->SBUF copies round to float32r: 1 cycle/row matmul
    speed with ~13-bit mantissa, measured score RMS err 3e-3); cast V to fp16.
  Flat q-tile loop interleaving the two batches (16 slots), with the score
    matmul emitted two slots ahead of its softmax consumer so the PE never
    drains. Per slot:
      mm1: S = QT.T @ VT into PSUM [128, 1024], per-512 halves; DVE
        reduce_max(negate) per half overlaps the other half's matmuls.
      ACT exp(S - max) per half -> fp16 E with fused row-sum; DMA xbar
        block-transpose per half (dest[p, kt, j] = E[j, kt*128+p]) so the
        second matmul can start after the first half.
      mm2: C = ET.T @ V_fp16 into PSUM; attn = E*(1/Z) (ACT copy+scale),
        context = C*(1/Z) (DVE), stores on the scalar HWDGE ring to keep the
        sync ring free for the xbar transposes.
"""
import sys

sys.path.insert(0, "/opt/trn_rl_repo")

import collections
from contextlib import ExitStack

import numpy as np

import concourse.bass as bass
import concourse.tile as tile
from concourse import mybir
from concourse.bass_utils import run_bass_kernel_spmd

F32 = mybir.dt.float32
F32R = mybir.dt.float32r
F16 = mybir.dt.float16

N_CORES = 8
B, LQ, LKV, D = 16, 1024, 1024, 512
BPC = B // N_CORES  # batches per core
NQT = LQ // 128  # q-tiles per batch
NKT = LKV // 128  # k-tiles per batch
NDT = D // 128  # d-tiles


# --- post-Tile pass: hardware wait-slot limits -------------------------------
# Engine instructions carry a single hardware semaphore-wait slot; Tile's
# sem-assigner sometimes emits more. Hoist excess waits onto single-wait NOPs
# spliced immediately before the instruction on the same engine (the NX
# sequencer dispatches in order, so the NOPs block until the sems clear).
_WAIT_LIMITS = collections.defaultdict(lambda: 1)


def _fix_wait_limits(nc):
    n_fixed = 0
    for fn in nc.m.functions:
        for blk in fn.blocks:
            out = []
            for inst in blk.instructions:
                limit = _WAIT_LIMITS[type(inst).__name__]
                si = inst.sync_info
                if si is not None and si.on_wait and len(si.on_wait) > limit:
                    hoist = list(si.on_wait)[: len(si.on_wait) - limit]
                    keep = list(si.on_wait)[len(si.on_wait) - limit :]
                    for i, w in enumerate(hoist):
                        out.append(
                            mybir.InstNoOp(
                                name=f"{inst.name}-waitnop{i}",
                                engine=inst.engine,
                                sync_info=mybir.SyncInfo(on_wait=[w], on_update=[]),
                                bass_nofuse=True,
                            )
                        )
                    inst.sync_info = mybir.SyncInfo(
                        on_wait=keep, on_update=list(si.on_update or [])
                    )
                    n_fixed += 1
                out.append(inst)
            blk.instructions = out
    return n_fixed


def build():
    nc = bass.Bass("TRN2", target_bir_lowering=False, debug=False)
    q = nc.dram_tensor("query", [BPC, LQ, D], F32, kind="ExternalInput").ap()
    v = nc.dram_tensor("value", [BPC, LKV, D], F32, kind="ExternalInput").ap()
    iden = nc.dram_tensor("iden", [128, 128], F32, kind="ExternalInput").ap()
    ctx_out = nc.dram_tensor("context", [BPC, LQ, D], F32, kind="ExternalOutput").ap()
    attn_out = nc.dram_tensor("attn", [BPC, LQ, LKV], F32, kind="ExternalOutput").ap()

    with ExitStack() as ctx:
        tc = ctx.enter_context(tile.TileContext(nc))
        singles = ctx.enter_context(tc.tile_pool(name="singles", bufs=1))
        iop = ctx.enter_context(tc.tile_pool(name="io", bufs=2))
        tp = ctx.enter_context(tc.tile_pool(name="tp", bufs=2))
        ep = ctx.enter_context(tc.tile_pool(name="ep", bufs=3))
        sp = ctx.enter_context(tc.tile_pool(name="sp", bufs=12))
        # PSUM: S tiles 2 banks x 3 bufs = 6; transpose staging / C-psum share
        # the last 2 banks (prologue vs steady state, temporally disjoint).
        pss = ctx.enter_context(tc.tile_pool(name="pss", bufs=3, space="PSUM"))
        psc = ctx.enter_context(tc.tile_pool(name="psc", bufs=2, space="PSUM"))

        ident = singles.tile([128, 128], F32)
        nc.sync.dma_start(ident[:], iden)

        # ---- prologue: loads ------------------------------------------------
        qn, vn = {}, {}
        for b in range(BPC):
            qn[b] = iop.tile([128, NQT, D], F32, tag="qn", name=f"qn{b}")
            nc.sync.dma_start(qn[b][:], q[b].rearrange("(t p) d -> p t d", p=128))
            vn[b] = iop.tile([128, NKT, D], F32, tag="vn", name=f"vn{b}")
            nc.sync.dma_start(vn[b][:], v[b].rearrange("(t p) d -> p t d", p=128))

        # ---- prologue: all transposes + fp16 V casts ------------------------
        qt, vt, vh = {}, {}, {}
        copy_eng = 0
        for b in range(BPC):
            vh[b] = tp.tile([128, NKT, D], F16, tag="vh", name=f"vh{b}")
            for kt in range(NKT):
                nc.vector.tensor_copy(vh[b][:, kt, :], vn[b][:, kt, :])
            qt[b] = tp.tile([128, NDT, LQ], F32R, tag="qt", name=f"qt{b}")
            vt[b] = tp.tile([128, NDT, LKV], F32R, tag="vt", name=f"vt{b}")
            for src, dst in ((qn[b], qt[b]), (vn[b], vt[b])):
                for dt in range(NDT):
                    for g in range(2):
                        pst = psc.tile([128, 512], F32, tag="pb", name=f"pst{b}{dt}{g}")
                        for j in range(4):
                            blk = g * 4 + j
                            nc.tensor.transpose(
                                pst[:, j * 128 : (j + 1) * 128],
                                src[:, blk, dt * 128 : (dt + 1) * 128],
                                ident[:],
                            )
                        dslice = dst[:, dt, g * 512 : (g + 1) * 512]
                        # alternate copies between DVE and ACT
                        if copy_eng % 2 == 0:
                            nc.vector.tensor_copy(dslice, pst[:])
                        else:
                            nc.scalar.copy(dslice, pst[:])
                        copy_eng += 1

        # ---- steady state: flat q-slot loop, batches interleaved ------------
        slots = [(b, qb) for qb in range(NQT) for b in range(BPC)]
        spsum = {}
        chain_state = {}

        def emit_mm1(slot):
            b, qb = slot
            s = pss.tile([128, LKV], F32, tag="s", name=f"s{b}_{qb}")
            halves = []
            for kc in range(2):
                for dt in range(NDT):
                    nc.tensor.matmul(
                        s[:, kc * 512 : (kc + 1) * 512],
                        qt[b][:, dt, qb * 128 : (qb + 1) * 128],
                        vt[b][:, dt, kc * 512 : (kc + 1) * 512],
                        start=(dt == 0),
                        stop=(dt == NDT - 1),
                    )
                # negated per-half max, overlapping the other half's matmuls
                nm = sp.tile([128, 1], F32, tag=f"nm{kc}", name=f"nm{b}_{qb}_{kc}")
                nc.vector.reduce_max(
                    nm[:], s[:, kc * 512 : (kc + 1) * 512],
                    axis=mybir.AxisListType.X, negate=True,
                )
                halves.append(nm)
            spsum[slot] = (s, halves)

        def emit_chain(slot):
            """Per-half online softmax: exp with the half's own max, xbar
            transpose per half, then tiny per-row correction scales."""
            b, qb = slot
            s, nmh = spsum.pop(slot)
            e16 = ep.tile([128, LKV], F16, tag="e16", name=f"e16{b}_{qb}")
            et = ep.tile([128, NKT, 128], F16, tag="et", name=f"et{b}_{qb}")
            z = []
            for h in range(2):
                zh = sp.tile([128, 1], F32, tag=f"z{h}", name=f"z{b}_{qb}_{h}")
                nc.scalar.activation(
                    e16[:, h * 512 : (h + 1) * 512],
                    s[:, h * 512 : (h + 1) * 512],
                    mybir.ActivationFunctionType.Exp,
                    bias=nmh[h][:],
                    scale=1.0,
                    accum_out=zh[:],
                )
                nc.sync.dma_start(
                    et[:, h * 4 : (h + 1) * 4, :],
                    e16[:, h * 512 : (h + 1) * 512],
                    transpose=True,
                )
                z.append(zh)
            # correction scales: m = max(m0, m1); rz_h = exp(m_h - m) / Z
            # with Z = z0*exp(m0 - m) + z1*exp(m1 - m).  (nm_h = -m_h)
            nm = sp.tile([128, 1], F32, tag="nm", name=f"nm{b}_{qb}")
            nc.vector.tensor_tensor(
                nm[:], nmh[0][:], nmh[1][:], op=mybir.AluOpType.min
            )
            eh, th = [], []
            for h in range(2):
                d = sp.tile([128, 1], F32, tag=f"d{h}", name=f"d{b}_{qb}_{h}")
                nc.vector.tensor_sub(d[:], nm[:], nmh[h][:])
                e = sp.tile([128, 1], F32, tag=f"e{h}", name=f"e{b}_{qb}_{h}")
                nc.scalar.activation(
                    e[:], d[:], mybir.ActivationFunctionType.Exp
                )
                t = sp.tile([128, 1], F32, tag=f"t{h}", name=f"t{b}_{qb}_{h}")
                nc.vector.tensor_mul(t[:], z[h][:], e[:])
                eh.append(e)
                th.append(t)
            zsum = sp.tile([128, 1], F32, tag="zsum", name=f"zs{b}_{qb}")
            nc.vector.tensor_add(zsum[:], th[0][:], th[1][:])
            rz = sp.tile([128, 1], F32, tag="rz", name=f"rz{b}_{qb}")
            nc.vector.reciprocal(rz[:], zsum[:])
            rzh = []
            for h in range(2):
                r = sp.tile([128, 1], F32, tag=f"rz{h}", name=f"rzh{b}_{qb}_{h}")
                nc.vector.tensor_mul(r[:], eh[h][:], rz[:])
                rzh.append(r)
            chain_state[slot] = (e16, et, rzh)

        def emit_mm2(slot):
            b, qb = slot
            e16, et, rzh = chain_state[slot]
            us = []
            for h in range(2):
                u = psc.tile([128, D], F32, tag="pb", name=f"u{b}_{qb}_{h}")
                for j in range(4):
                    kt = h * 4 + j
                    nc.tensor.matmul(
                        u[:],
                        et[:, kt, :],
                        vh[b][:, kt, :],
                        start=(j == 0),
                        stop=(j == 3),
                    )
                us.append(u)
            chain_state[slot] = (e16, et, rzh, us)

        def emit_cpath(slot):
            b, qb = slot
            e16, et, rzh, us = chain_state[slot]
            ca = ep.tile([128, D], F32, tag="ca", name=f"ca{b}_{qb}")
            nc.vector.tensor_scalar_mul(ca[:], us[0][:], rzh[0][:])
            c32 = ep.tile([128, D], F32, tag="c32", name=f"c32{b}_{qb}")
            nc.vector.scalar_tensor_tensor(
                c32[:], us[1][:], rzh[1][:], ca[:],
                op0=mybir.AluOpType.mult, op1=mybir.AluOpType.add,
            )
            nc.gpsimd.dma_start(ctx_out[b, qb * 128 : (qb + 1) * 128, :], c32[:])

        def emit_apath(slot):
            b, qb = slot
            e16, et, rzh, us = chain_state.pop(slot)
            a32 = ep.tile([128, LKV], F32, tag="a32", name=f"a32{b}_{qb}")
            for h in range(2):
                nc.scalar.activation(
                    a32[:, h * 512 : (h + 1) * 512],
                    e16[:, h * 512 : (h + 1) * 512],
                    mybir.ActivationFunctionType.Copy,
                    scale=rzh[h][:],
                )
            nc.gpsimd.dma_start(attn_out[b, qb * 128 : (qb + 1) * 128, :], a32[:])

        n = len(slots)
        emit_mm1(slots[0])
        for i in range(n + 1):
            if i >= 1:
                emit_mm2(slots[i - 1])
                emit_cpath(slots[i - 1])
            if i < n:
                emit_chain(slots[i])
            if i >= 1:
                emit_apath(slots[i - 1])
            if i + 1 < n:
                emit_mm1(slots[i + 1])

    _fix_wait_limits(nc)
    return nc


_NC = None


def _get_nc():
    global _NC
    if _NC is None:
        _NC = build()
    return _NC


_IDEN = np.eye(128, dtype=np.float32)


def kernel(query: np.ndarray, value: np.ndarray):
    query = np.ascontiguousarray(query, dtype=np.float32)
    value = np.ascontiguousarray(value, dtype=np.float32)
    nc = _get_nc()
    in_maps = [
        {
            "query": query[c * BPC : (c + 1) * BPC],
            "value": value[c * BPC : (c + 1) * BPC],
            "iden": _IDEN,
        }
        for c in range(N_CORES)
    ]
    res = run_bass_kernel_spmd(nc, in_maps, core_ids=list(range(N_CORES)))
    context = np.concatenate([r["context"] for r in res.results], axis=0)
    attn = np.concatenate([r["attn"] for r in res.results], axis=0)
    return context, attn
